# revision 47
# baseline (speedup 1.0000x reference)
"""Trainium2 Bass kernel for nn_Graph_CNN_Feat_Mesh (Chebyshev GNN decoder).

Strategy (per-core, data-parallel over batch B=256 -> 32/core):
  - All spmms are dense matmuls on the tensor engine (PE) in bf16:
      y = A + L @ (B + L @ (2C)),  A/B/C = feature-space linears of the input.
    L is densified on host; for up4-preceded layers the replication is folded
    into LU = L @ U (contracting the small pre-upsample vertex space).
  - Internal vertex order is relabeled v' = w + Vsp*r (r-major blocks) so the
    up4 replication is 4 contiguous block copies; the final output is
    unscrambled in numpy.
  - BatchNorm (training mode, global batch stats) is exact: per-core partial
    (sum, sumsq) are AllGather'd across the 8 cores, reduced on-chip.  The
    affine+relu is refactored as relu(s*x+t) = s*relu(x + t/s) (s>0): the
    bias-relu is a single fused op (folded into the up4-replication copies
    where possible) and the scale s is folded into the next layer's weights.
  - FC head runs in bf16 with fp32 PSUM accumulation.
"""

import numpy as np

B = 256
NCORES = 8
BL = B // NCORES  # 32
EPS = 1e-5

_CACHE = {}


def _split_W(W):
    W = np.asarray(W, np.float32)
    return W[:, 0::3], W[:, 1::3], W[:, 2::3]


def _dense_L(rows, cols, vals, V):
    L = np.zeros((V, V), np.float32)
    np.add.at(L, (np.asarray(rows), np.asarray(cols)), np.asarray(vals, np.float32))
    return L


def _pad_rows(a, m):
    if a.shape[0] % m == 0:
        return a
    p = m - a.shape[0] % m
    return np.concatenate([a, np.zeros((p,) + a.shape[1:], a.dtype)], 0)


def _perm(V, Vsp):
    # v' = w + Vsp*r  ->  canonical v = 4*w + r
    vp = np.arange(V)
    return 4 * (vp % Vsp) + vp // Vsp


def _perm2():
    # c2/c3 dest order composes with the P1-relabeled 320 source space
    vp = np.arange(1280)
    return 4 * _perm(320, 80)[vp % 320] + vp // 320


class _LCfg:
    def __init__(self, name, Vsp, V, Fin, Fout, up4, bn):
        self.name = name
        self.Vsp = Vsp      # source vertex space of C-linear (pre-up4)
        self.V = V          # output vertex count
        self.Fin = Fin
        self.Fout = Fout
        self.G = 128 // Fin          # batches packed on partitions at input
        self.nG = BL // self.G
        self.GF = self.G * Fout      # N of one B/C/A-linear matmul
        self.Gp = 128 // Fout if Fout in (32, 64) else None
        self.nGp = BL // self.Gp if self.Gp else None
        self.up4 = up4
        self.bn = bn
        self.nVt = (V + 127) // 128
        self.nVsp = (Vsp + 127) // 128
        self.BF = BL * Fout          # free width of V-layout per vtile

    def vts(self, t):
        return min(128, self.V - t * 128)

    def sps(self, s):
        return min(128, self.Vsp - s * 128)


CFGS = [
    _LCfg("c0", 80, 320, 64, 64, True, True),
    _LCfg("c1", 320, 320, 64, 32, False, True),
    _LCfg("c2", 320, 1280, 32, 32, True, True),
    _LCfg("c3", 1280, 1280, 32, 3, False, False),
]


def _wbd(W, G, Fin, Fout, which):
    """Block-diagonal rhs weight [128, G*Fout] for the fused linear.
    which: 'A' -> W0 - W2, 'B' -> W1, 'C' -> 2*W2.  col = j*Fout + c."""
    W0, W1, W2 = _split_W(W)
    M = {"A": W0 - W2, "B": W1, "C": 2.0 * W2}[which]  # [Fout, Fin]
    out = np.zeros((128, G * Fout), np.float32)
    for j in range(G):
        out[j * Fin:(j + 1) * Fin, j * Fout:(j + 1) * Fout] = M.T
    return out


# const-blob column offsets (f32 [128, CW])
_CB_FC1B = 0           # [128, 4]
_CB_SEL64 = 4          # [128, 64]
_CB_SEL32 = 68         # [128, 32]
_CB_SELT64 = 100       # [64, 128]
_CB_SELT32 = 228       # [32, 128]
_CB_GB0 = 356          # [1, 128]
_CB_GB1 = 484          # [1, 64]
_CB_GB2 = 548          # [1, 64]
_CB_W = 612


def _build_host(inputs):
    import ml_dtypes
    bf = ml_dtypes.bfloat16
    f32 = np.float32
    d = {}
    d["xT"] = np.ascontiguousarray(np.asarray(inputs["x"], f32).T).astype(bf)
    d["fc1wT"] = np.ascontiguousarray(np.asarray(inputs["fc1_w"], f32).T).astype(bf)
    d["fc2wT"] = np.ascontiguousarray(np.asarray(inputs["fc2_w"], f32).T).astype(bf)

    L1 = _dense_L(inputs["L1_rows"], inputs["L1_cols"], inputs["L1_vals"], 320)
    L2 = _dense_L(inputs["L2_rows"], inputs["L2_cols"], inputs["L2_vals"], 1280)
    U1 = np.repeat(np.eye(80, dtype=f32), 4, axis=0)    # [320, 80]
    U2 = np.repeat(np.eye(320, dtype=f32), 4, axis=0)   # [1280, 320]
    P1 = _perm(320, 80)
    P2 = _perm2()
    L1p = L1[P1][:, P1]
    L2p = L2[P2][:, P2]
    d["LU0"] = _pad_rows(np.ascontiguousarray((L1 @ U1)[P1, :].T), 128).astype(bf)
    d["LT1"] = _pad_rows(np.ascontiguousarray(L1p.T), 128).astype(bf)   # [384,320]
    d["LU2"] = _pad_rows(
        np.ascontiguousarray((L2 @ U2)[P2][:, P1].T), 128).astype(bf)   # [384,1280]
    d["LT2"] = np.ascontiguousarray(L2p.T).astype(bf)                   # [1280,1280]

    # all cheby weights in one blob [128, sum(GF)*3]
    Wn = {"c0": "cl0_w", "c1": "cl1_w", "c2": "cl2_w", "c3": "cl3_w"}
    wcols = []
    offs = {}
    off = 0
    for cfg in CFGS:
        W = np.asarray(inputs[Wn[cfg.name]], f32)
        for which in "ABC":
            wcols.append(_wbd(W, cfg.G, cfg.Fin, cfg.Fout, which))
            offs[f"{which}{cfg.name}"] = (off, cfg.GF)
            off += cfg.GF
    # b3 bias row [1, 96]: col = g*12 + j*3 + fo -> b3[fo]
    b3 = np.asarray(inputs["cl3_b"], f32)
    b3row = np.zeros((128, 96), f32)
    b3row[0, :] = np.tile(b3, 32)
    wcols.append(b3row)
    offs["b3row"] = (off, 96)
    off += 96
    d["wcat"] = np.concatenate(wcols, axis=1).astype(bf)
    d["_woffs"] = offs

    cb = np.zeros((128, _CB_W), f32)
    cb[:, _CB_FC1B:_CB_FC1B + 4] = np.asarray(inputs["fc1_b"], f32).reshape(4, 128).T
    for F, base in [(64, _CB_SEL64), (32, _CB_SEL32)]:
        for j in range(128 // F):
            cb[j * F:(j + 1) * F, base:base + F] += np.eye(F, dtype=f32)
    cb[:64, _CB_SELT64:_CB_SELT64 + 128] = cb[:, _CB_SEL64:_CB_SEL64 + 64].T
    cb[:32, _CB_SELT32:_CB_SELT32 + 128] = cb[:, _CB_SEL32:_CB_SEL32 + 32].T
    for base, (g, b_) in [(_CB_GB0, ("bn0_g", "bn0_b")),
                          (_CB_GB1, ("bn1_g", "bn1_b")),
                          (_CB_GB2, ("bn2_g", "bn2_b"))]:
        gb = np.concatenate([np.asarray(inputs[g], f32), np.asarray(inputs[b_], f32)])
        cb[0, base:base + len(gb)] = gb
    d["cblob"] = cb
    return d


def _build_nc(woffs):
    import sys
    for p in ("/opt/trn_rl_repo", "/opt/trn_rl_repo/concourse"):
        if p not in sys.path:
            sys.path.insert(0, p)
    import concourse.bass as bass  # noqa
    import concourse.mybir as mybir
    import concourse.tile as tile
    from concourse import bacc
    from concourse.masks import make_identity

    f32 = mybir.dt.float32
    bf16 = mybir.dt.bfloat16
    AF = mybir.ActivationFunctionType
    ALU = mybir.AluOpType

    nc = bacc.Bacc(None, target_bir_lowering=False)

    xT = nc.dram_tensor("xT", [2048, BL], bf16, kind="ExternalInput")
    fc1wT = nc.dram_tensor("fc1wT", [2048, 512], bf16, kind="ExternalInput")
    fc2wT = nc.dram_tensor("fc2wT", [512, 5120], bf16, kind="ExternalInput")
    LU0 = nc.dram_tensor("LU0", [128, 320], bf16, kind="ExternalInput")
    LT1 = nc.dram_tensor("LT1", [384, 320], bf16, kind="ExternalInput")
    LU2 = nc.dram_tensor("LU2", [384, 1280], bf16, kind="ExternalInput")
    LT2 = nc.dram_tensor("LT2", [1280, 1280], bf16, kind="ExternalInput")
    WCW = woffs["b3row"][0] + 96
    wcat = nc.dram_tensor("wcat", [128, WCW], bf16, kind="ExternalInput")
    cblob = nc.dram_tensor("cblob", [128, _CB_W], f32, kind="ExternalInput")
    ydram = nc.dram_tensor("y", [BL, 1280 * 3], f32, kind="ExternalOutput")

    with tile.TileContext(nc) as tc:
        with (
            tc.tile_pool(name="const", bufs=1) as constp,
            tc.tile_pool(name="wpool", bufs=1) as wpool,
            tc.tile_pool(name="poolA", bufs=2) as poolA,
            tc.tile_pool(name="poolB", bufs=2) as poolB,
            tc.tile_pool(name="poolC", bufs=1) as poolC,
            tc.tile_pool(name="misc", bufs=1) as miscp,
            tc.tile_pool(name="outp", bufs=1) as outp,
            tc.tile_pool(name="psA", bufs=3, space="PSUM") as psA,
            tc.tile_pool(name="psB", bufs=2, space="PSUM") as psB,
            tc.tile_pool(name="psT", bufs=1, space="PSUM") as psT,
            tc.tile_pool(name="dram", bufs=1, space="DRAM") as dramp,
        ):
            # ---------- input DMAs (merged; issue order = priority) ----------
            xT_sb = miscp.tile([128, 16 * BL], bf16, tag="xT")
            nc.sync.dma_start(
                xT_sb[:].rearrange("p (k b) -> p k b", b=BL),
                xT[:].rearrange("(k p) b -> p k b", p=128))
            fc1w_sb = poolA.tile([128, 16 * 512], bf16, tag="A", name="fc1w")
            for h in range(2):
                nc.sync.dma_start(
                    fc1w_sb[:, h * 4096:(h + 1) * 4096]
                    .rearrange("p (k m) -> p k m", m=512),
                    fc1wT[h * 1024:(h + 1) * 1024, :]
                    .rearrange("(k p) m -> p k m", p=128))
            cb_sb = constp.tile([128, _CB_W], f32, tag="cblob")
            nc.sync.dma_start(cb_sb[:], cblob[:])
            wch = []
            for hc in range(8):
                t_ = poolB.tile([128, 4 * 640], bf16, tag="B", name=f"wch{hc}")
                nc.sync.dma_start(
                    t_[:].rearrange("p (k m) -> p k m", m=640),
                    fc2wT[:, hc * 640:(hc + 1) * 640]
                    .rearrange("(k p) m -> p k m", p=128))
                wch.append(t_)

            wcat_sb = wpool.tile([128, WCW], bf16, tag="wcat")
            nc.sync.dma_start(wcat_sb[:], wcat[:])

            LUT, LT = {}, {}
            t_ = wpool.tile([128, 320], bf16, tag="LU0")
            nc.sync.dma_start(t_[:], LU0[0:128, :])
            LUT["c0"] = t_
            t_ = wpool.tile([128, 3 * 320], bf16, tag="LT1")
            nc.sync.dma_start(
                t_[:].rearrange("p (s n) -> p s n", n=320),
                LT1[:].rearrange("(s p) n -> p s n", p=128))
            LT["c0"] = LT["c1"] = LUT["c1"] = t_
            t_ = wpool.tile([128, 3 * 1280], bf16, tag="LU2")
            nc.sync.dma_start(
                t_[:].rearrange("p (s n) -> p s n", n=1280),
                LU2[:].rearrange("(s p) n -> p s n", p=128))
            LUT["c2"] = t_
            lt2_sb = wpool.tile([128, 10 * 1280], bf16, tag="LT2")
            for h in range(2):
                nc.sync.dma_start(
                    lt2_sb[:, h * 5 * 1280:(h + 1) * 5 * 1280]
                    .rearrange("p (s n) -> p s n", n=1280),
                    LT2[h * 640:(h + 1) * 640, :]
                    .rearrange("(s p) n -> p s n", p=128))
            LT["c2"] = LT["c3"] = LUT["c3"] = lt2_sb

            W_sb = {}
            for cfg in CFGS:
                for w in "ABC":
                    o, gf = woffs[f"{w}{cfg.name}"]
                    W_sb[f"{w}{cfg.name}"] = wcat_sb[:, o:o + gf]
            o, _ = woffs["b3row"]
            b3row = wcat_sb[:1, o:o + 96]

            # ---------- small consts ----------
            ident_b = constp.tile([128, 128], bf16, tag="identb")
            make_identity(nc, ident_b[:])
            ident_f = constp.tile([128, 128], f32, tag="identf")
            make_identity(nc, ident_f[:])
            eps_t = constp.tile([1, 1], f32, tag="eps")
            nc.gpsimd.memset(eps_t[:], EPS)
            ones8 = constp.tile([8, 1], f32, tag="ones8")
            nc.gpsimd.memset(ones8[:], 1.0)
            onesr = constp.tile([1, 128], bf16, tag="onesr")
            nc.gpsimd.memset(onesr[:], 1.0)
            fc1b_sb = cb_sb[:, _CB_FC1B:_CB_FC1B + 4]
            sel_sb = {64: cb_sb[:, _CB_SEL64:_CB_SEL64 + 64],
                      32: cb_sb[:, _CB_SEL32:_CB_SEL32 + 32]}
            selT_sb = {64: cb_sb[:64, _CB_SELT64:_CB_SELT64 + 128],
                       32: cb_sb[:32, _CB_SELT32:_CB_SELT32 + 128]}
            gb_sb = [cb_sb[:1, _CB_GB0:_CB_GB0 + 128],
                     cb_sb[:1, _CB_GB1:_CB_GB1 + 64],
                     cb_sb[:1, _CB_GB2:_CB_GB2 + 64]]

            # ================= FC head (bf16) =================
            h1T = miscp.tile([128, 4 * BL], bf16, tag="h1T")
            ps1 = psA.tile([128, 512], f32, tag="a")
            for mt in range(4):
                for kt in range(16):
                    nc.tensor.matmul(
                        ps1[:, mt * BL:(mt + 1) * BL],
                        fc1w_sb[:, kt * 512 + mt * 128: kt * 512 + (mt + 1) * 128],
                        xT_sb[:, kt * BL:(kt + 1) * BL],
                        start=(kt == 0), stop=(kt == 15))
                nc.scalar.activation(
                    h1T[:, mt * BL:(mt + 1) * BL], ps1[:, mt * BL:(mt + 1) * BL],
                    AF.Relu, bias=fc1b_sb[:, mt:mt + 1])

            # fc2: psum partition = (v0%2)*64+f, col = mi*BL+b ; chan c = v0*64+f.
            # dest: XF0[(b%2)*64+f, (b//2)*80 + v0],  v0 = 2*(mc*10+mi)+p0
            XF0 = poolC.tile([128, 16 * 80], bf16, tag="XF0")
            for hc in range(8):
                ps2 = psB.tile([128, 5 * BL], f32, tag="big")
                for mi in range(5):
                    for kt in range(4):
                        nc.tensor.matmul(
                            ps2[:, mi * BL:(mi + 1) * BL],
                            wch[hc][:, kt * 640 + mi * 128:
                                 kt * 640 + (mi + 1) * 128],
                            h1T[:, kt * BL:(kt + 1) * BL],
                            start=(kt == 0), stop=(kt == 3))
                src4 = ps2[:].rearrange("p (i g j) -> p i g j", g=16, j=2)
                dst4 = XF0[:].rearrange("p (g u q) -> p g u q", u=40, q=2)
                for p0 in range(2):
                    for j in range(2):
                        nc.scalar.activation(
                            dst4[j * 64:(j + 1) * 64, :,
                                 hc * 5:(hc + 1) * 5, p0]
                            .rearrange("p g i -> p i g"),
                            src4[p0 * 64:(p0 + 1) * 64, :, :, j],
                            AF.Copy)

            # ================= cheby layers =================
            XF_cur = XF0
            ar_idx = 0

            for li, cfg in enumerate(CFGS):
                V, Vsp, F = cfg.V, cfg.Vsp, cfg.Fout
                BF = cfg.BF
                last = cfg.name == "c3"
                first = li == 0

                # --- up4 replication (fused bias-relu except c0) ---
                # XFrep[:, g*V + r*Vsp + w] = act(XF_cur[:, g*Vsp + w])
                if cfg.up4:
                    XFrep = poolA.tile([128, cfg.nG * V], bf16, tag="A",
                                       name=f"XFrep{li}")
                    ng2 = cfg.nG // 2
                    s_r = XF_cur[:].rearrange("p (g w) -> p g w", w=Vsp)
                    d_r = XFrep[:].rearrange("p (g r w) -> p r g w", r=4, w=Vsp)
                    # r=0 feeds the C-linear: split across both engines first
                    for r, gs in [(0, slice(0, ng2)), (0, slice(ng2, cfg.nG)),
                                  (1, None), (2, None), (3, None)]:
                        gsl = gs if gs is not None else slice(0, cfg.nG)
                        dst_ = d_r[:, r, gsl]
                        src_ = s_r[:, gsl]
                        if first:
                            if gs == slice(ng2, cfg.nG) or r == 2:
                                nc.scalar.activation(dst_, src_, AF.Copy)
                            else:
                                nc.vector.tensor_copy(dst_, src_)
                        else:
                            tp = stc_prev
                            if gs == slice(ng2, cfg.nG):
                                nc.scalar.activation(
                                    dst_, src_, AF.Relu, bias=tp[:, 1:2])
                            else:
                                nc.vector.tensor_scalar(
                                    out=dst_, in0=src_,
                                    scalar1=tp[:, 1:2], scalar2=0.0,
                                    op0=ALU.add, op1=ALU.max)
                    XFin = XFrep
                else:
                    XFin = XF_cur  # relu already applied in prev epilogue

                # --- C linear (in Vsp space; reads XFin r=0 block for up4) ---
                XC = poolC.tile([128, cfg.nVsp * BL * F], bf16, tag="XC")
                gpack = max(1, 512 // cfg.GF)
                rw = V if cfg.up4 else Vsp
                # c0's C-linear can read XF0 directly (no relu fused in rep)
                csrc, crw = ((XF_cur, Vsp) if (cfg.up4 and first)
                             else (XFin, rw))
                cpi = 0
                BLF = BL * F
                if BLF <= 256:
                    # pack 2 source tiles per psum tile (c3: BLF=96)
                    for s0 in range(0, cfg.nVsp, 2):
                        pc = psA.tile([128, 512], f32, tag="a")
                        for ds in range(2):
                            s = s0 + ds
                            for g in range(cfg.nG):
                                nc.tensor.matmul(
                                    pc[:128, ds * BLF + g * cfg.GF:
                                       ds * BLF + (g + 1) * cfg.GF],
                                    csrc[:, g * crw + s * 128:
                                         g * crw + s * 128 + 128],
                                    W_sb[f"C{cfg.name}"],
                                    start=True, stop=True)
                        dst_ = XC[:, s0 * BLF:(s0 + 2) * BLF]
                        if cpi % 3 == 2:
                            nc.vector.tensor_copy(dst_, pc[:, :2 * BLF])
                        else:
                            nc.scalar.activation(dst_, pc[:, :2 * BLF],
                                                 AF.Copy)
                        cpi += 1
                else:
                    for s in range(cfg.nVsp):
                        ssz = cfg.sps(s)
                        for g0 in range(0, cfg.nG, gpack):
                            gn = min(gpack, cfg.nG - g0)
                            pc = psA.tile([128, 512], f32, tag="a")
                            for gi in range(gn):
                                g = g0 + gi
                                nc.tensor.matmul(
                                    pc[:ssz, gi * cfg.GF:(gi + 1) * cfg.GF],
                                    csrc[:, g * crw + s * 128:
                                         g * crw + s * 128 + ssz],
                                    W_sb[f"C{cfg.name}"],
                                    start=True, stop=True)
                            dst_ = XC[:ssz, s * BL * F + g0 * cfg.GF:
                                      s * BL * F + (g0 + gn) * cfg.GF]
                            if cpi % 3 == 2:
                                nc.vector.tensor_copy(dst_,
                                                      pc[:ssz, :gn * cfg.GF])
                            else:
                                nc.scalar.activation(dst_,
                                                     pc[:ssz, :gn * cfg.GF],
                                                     AF.Copy)
                            cpi += 1

                # --- inner = LU @ (2C) + B ;  y = L @ inner + A (+bias c3) ---
                Xin = poolB.tile([128, cfg.nVt * BF], bf16, tag="B",
                                 name=f"Xin{li}")
                if not last:
                    XFn = poolA.tile([128, cfg.nGp * V], bf16, tag="A",
                                     name=f"XFn{li}")
                    nch = (V + 511) // 512
                    bnst = miscp.tile([128, cfg.nGp * nch * 6], f32,
                                      tag="bnst")
                else:
                    ytile = poolC.tile([128, cfg.nVt * BF], bf16, tag="YT")
                    och = outp.tile([BL, 3840], f32, tag="out")

                def out_tail(t2):
                    # c3: transposes of tiles t2, t2+1 -> och copies
                    pt = psT.tile([128, 512], bf16, tag="tr")
                    for dt_ in range(2):
                        t = t2 + dt_
                        nc.tensor.transpose(
                            pt[:96, dt_ * 128:(dt_ + 1) * 128],
                            ytile[:128, t * BF:(t + 1) * BF],
                            ident_b[:128, :128])
                    for fo in range(3):
                        dst = och[:].rearrange("b (v f) -> b v f", f=3)[
                            :, t2 * 128:(t2 + 2) * 128, fo]
                        if fo % 2 == 0:
                            nc.vector.tensor_copy(dst, pt[fo * 32:fo * 32 + 32, :256])
                        else:
                            nc.scalar.activation(
                                dst, pt[fo * 32:fo * 32 + 32, :256], AF.Copy)

                # ---- phase 0 (V-form): Xin[t, (g,j,fo)] = LU @ 2C + B ----
                srcL = LUT[cfg.name]
                nS = cfg.nVsp
                if BF <= 256:
                    # pack 2 dest tiles per psum tile (c3: BF=96)
                    for t0 in range(0, cfg.nVt, 2):
                        pi = psB.tile([128, 512], f32, tag="big")
                        for dt_ in range(2):
                            t = t0 + dt_
                            base = dt_ * BF
                            for s in range(nS):
                                ssz = cfg.sps(s)
                                nc.tensor.matmul(
                                    pi[:128, base:base + BF],
                                    srcL[:ssz, s * V + t * 128:
                                         s * V + t * 128 + 128],
                                    XC[:ssz, s * BL * F:(s + 1) * BL * F],
                                    start=(s == 0), stop=False,
                                    skip_group_check=True)
                            for g in range(cfg.nG):
                                nc.tensor.matmul(
                                    pi[:128, base + g * cfg.GF:
                                       base + (g + 1) * cfg.GF],
                                    XFin[:, g * V + t * 128:
                                         g * V + t * 128 + 128],
                                    W_sb[f"B{cfg.name}"],
                                    start=False, stop=True,
                                    skip_group_check=True)
                        if cpi % 3 == 2:
                            nc.vector.tensor_copy(
                                Xin[:, t0 * BF:(t0 + 2) * BF],
                                pi[:, :2 * BF])
                        else:
                            nc.scalar.activation(
                                Xin[:, t0 * BF:(t0 + 2) * BF],
                                pi[:, :2 * BF], AF.Copy)
                        cpi += 1
                else:
                    for t in range(cfg.nVt):
                        vsz = cfg.vts(t)
                        for pc0 in range(0, BF, 1024):
                            pw = min(1024, BF - pc0)
                            pi = psB.tile([128, max(pw, 512)], f32, tag="big")
                            for nk in range(0, pw, 512):
                                n0 = pc0 + nk
                                n1 = min(n0 + 512, pc0 + pw)
                                for s in range(nS):
                                    ssz = cfg.sps(s)
                                    nc.tensor.matmul(
                                        pi[:vsz, n0 - pc0:n1 - pc0],
                                        srcL[:ssz, s * V + t * 128:
                                             s * V + t * 128 + vsz],
                                        XC[:ssz, s * BL * F + n0:
                                           s * BL * F + n1],
                                        start=(s == 0), stop=False,
                                        skip_group_check=True)
                                for g in range(n0 // cfg.GF,
                                               (n1 + cfg.GF - 1) // cfg.GF):
                                    nc.tensor.matmul(
                                        pi[:vsz, g * cfg.GF - pc0:
                                           (g + 1) * cfg.GF - pc0],
                                        XFin[:, g * V + t * 128:
                                             g * V + t * 128 + vsz],
                                        W_sb[f"B{cfg.name}"],
                                        start=False, stop=True,
                                        skip_group_check=True)
                            if cpi % 3 == 2:
                                nc.vector.tensor_copy(
                                    Xin[:vsz, t * BF + pc0:
                                        t * BF + pc0 + pw],
                                    pi[:vsz, :pw])
                            else:
                                nc.scalar.activation(
                                    Xin[:vsz, t * BF + pc0:
                                        t * BF + pc0 + pw],
                                    pi[:vsz, :pw], AF.Copy)
                            cpi += 1

                # ---- phase 1 ----
                srcL = LT[cfg.name]
                nS = cfg.nVt
                if not last:
                    # transposed form: XFn[:, q*V+v] = sum_s Xin[s,q]^T L^T
                    #                                + W_A^T XFin_g  (F-layout)
                    QG = 128 // cfg.GF
                    nQ = cfg.nG // QG
                    for q in range(nQ):
                        for ci, v0 in enumerate(range(0, V, 512)):
                            vw = min(512, V - v0)
                            pi = psA.tile([128, 512], f32, tag="a")
                            for s in range(nS):
                                ssz = cfg.vts(s)
                                nc.tensor.matmul(
                                    pi[:, :vw],
                                    Xin[:ssz, s * BF + q * 128:
                                        s * BF + (q + 1) * 128],
                                    srcL[:ssz, s * V + v0:s * V + v0 + vw],
                                    start=(s == 0), stop=(s == nS - 1),
                                    skip_group_check=True)
                                if s == 0:
                                    for gi in range(QG):
                                        g = q * QG + gi
                                        nc.tensor.matmul(
                                            pi[gi * cfg.GF:
                                               (gi + 1) * cfg.GF, :vw],
                                            W_sb[f"A{cfg.name}"],
                                            XFin[:, g * V + v0:
                                                 g * V + v0 + vw],
                                            start=False, stop=False,
                                            skip_group_check=True)
                            if cpi % 3 == 2:
                                nc.vector.tensor_copy(
                                    XFn[:, q * V + v0:q * V + v0 + vw],
                                    pi[:, :vw])
                            else:
                                nc.scalar.activation(
                                    XFn[:, q * V + v0:q * V + v0 + vw],
                                    pi[:, :vw], AF.Copy)
                            cpi += 1
                            nc.vector.bn_stats(
                                bnst[:, (q * nch + ci) * 6:
                                     (q * nch + ci + 1) * 6],
                                XFn[:, q * V + v0:q * V + v0 + vw])
                else:
                    # V-form with bias, reorder copy, output staging
                    # (pairs of tiles per psum tile; BF=96)
                    for t0 in range(0, cfg.nVt, 2):
                        pi = psA.tile([128, 512], f32, tag="a")
                        for dt_ in range(2):
                            t = t0 + dt_
                            base = dt_ * BF
                            nc.tensor.matmul(
                                pi[:128, base:base + BF],
                                onesr[:, :128], b3row[:, :BF],
                                start=True, stop=False,
                                skip_group_check=True)
                            for s in range(nS):
                                ssz = cfg.vts(s)
                                nc.tensor.matmul(
                                    pi[:128, base:base + BF],
                                    srcL[:ssz, s * V + t * 128:
                                         s * V + t * 128 + 128],
                                    Xin[:ssz, s * BF:(s + 1) * BF],
                                    start=False, stop=False,
                                    skip_group_check=True)
                            for g in range(cfg.nG):
                                nc.tensor.matmul(
                                    pi[:128, base + g * cfg.GF:
                                       base + (g + 1) * cfg.GF],
                                    XFin[:, g * V + t * 128:
                                         g * V + t * 128 + 128],
                                    W_sb[f"A{cfg.name}"],
                                    start=False, stop=True,
                                    skip_group_check=True)
                        # reorder (b,fo) -> (fo,b) for output staging
                        dst_ = ytile[:, t0 * BF:(t0 + 2) * BF] \
                            .rearrange("p (T c b) -> p T c b", T=2, b=BL)
                        src_ = pi[:, :2 * BF] \
                            .rearrange("p (T b c) -> p T c b", T=2, c=3)
                        if t0 % 4 == 0:
                            nc.vector.tensor_copy(dst_, src_)
                        else:
                            nc.scalar.activation(dst_, src_, AF.Copy)
                        if t0 >= 2:
                            out_tail(t0 - 2)
                            if t0 == 4:
                                nc.sync.dma_start(ydram[:, :1536],
                                                  och[:, :1536])
                            elif t0 == 8:
                                nc.sync.dma_start(ydram[:, 1536:3072],
                                                  och[:, 1536:3072])
                    out_tail(cfg.nVt - 2)

                if cfg.bn:
                    # --- BN stats -> AllGather -> s, t' ---
                    Gp = cfg.Gp
                    FD = cfg.nGp * V
                    aggr = miscp.tile([128, 2], f32, tag="aggr")
                    nc.vector.bn_aggr(
                        aggr[:], bnst[:].rearrange("p (c s) -> p c s", s=6))
                    part = miscp.tile([128, 2], f32, tag="part")
                    nc.vector.tensor_tensor(
                        out=part[:, 1:2], in0=aggr[:, 0:1], in1=aggr[:, 0:1],
                        op=ALU.mult)
                    nc.vector.tensor_tensor(
                        out=part[:, 1:2], in0=part[:, 1:2], in1=aggr[:, 1:2],
                        op=ALU.add)
                    nc.vector.tensor_scalar_mul(part[:, 1:2], part[:, 1:2],
                                                float(FD))
                    nc.vector.tensor_scalar_mul(part[:, 0:1], aggr[:, 0:1],
                                                float(FD))
                    pst = psA.tile([128, 512], f32, tag="a")
                    nc.tensor.matmul(pst[:1, :F], part[:, 0:1], sel_sb[F],
                                     start=True, stop=True)
                    nc.tensor.matmul(pst[:1, F:2 * F], part[:, 1:2],
                                     sel_sb[F], start=True, stop=True)
                    stats_l = miscp.tile([1, 2 * F], f32, tag="statl")
                    nc.vector.tensor_copy(stats_l[:], pst[:1, :2 * F])
                    bin_ = dramp.tile([1, 2 * F], f32, tag=f"arin{ar_idx}")
                    bout = dramp.tile([8, 2 * F], f32, tag=f"arout{ar_idx}")
                    nc.sync.dma_start(bin_[:], stats_l[:])
                    nc.gpsimd.collective_compute(
                        "AllGather", ALU.bypass,
                        replica_groups=[list(range(NCORES))],
                        ins=[bin_.opt()], outs=[bout.opt()])
                    stats_g8 = miscp.tile([8, 2 * F], f32, tag="statg8")
                    nc.sync.dma_start(stats_g8[:], bout[:])
                    psg = psA.tile([128, 512], f32, tag="a", name="psg")
                    nc.tensor.matmul(psg[:1, :2 * F], ones8[:, 0:1],
                                     stats_g8[:], start=True, stop=True)
                    n_g = float(B * V)
                    # tmp: [0:F]=mu, [F:2F]=E[y^2]->var ; st: [0:F]=s, [F:2F]=t'
                    st = miscp.tile([1, 2 * F], f32, tag="st")
                    tmp = miscp.tile([1, 2 * F], f32, tag="sttmp")
                    mu2 = miscp.tile([1, F], f32, tag="mu2")
                    nc.vector.tensor_scalar_mul(tmp[:, :2 * F],
                                                psg[:1, :2 * F], 1.0 / n_g)
                    nc.vector.tensor_tensor(out=mu2[:], in0=tmp[:, 0:F],
                                            in1=tmp[:, 0:F], op=ALU.mult)
                    nc.vector.tensor_tensor(out=tmp[:, F:2 * F],
                                            in0=tmp[:, F:2 * F],
                                            in1=mu2[:], op=ALU.subtract)
                    # s = gamma / sqrt(var+eps)   (assumes gamma > 0)
                    nc.scalar.activation(st[:, 0:F], tmp[:, F:2 * F],
                                         AF.Sqrt, bias=eps_t[:])
                    nc.vector.reciprocal(st[:, 0:F], st[:, 0:F])
                    nc.vector.tensor_tensor(out=st[:, 0:F], in0=st[:, 0:F],
                                            in1=gb_sb[li][:, 0:F], op=ALU.mult)
                    # t' = beta / s - mu
                    nc.vector.reciprocal(mu2[:], st[:, 0:F])
                    nc.vector.tensor_tensor(out=mu2[:],
                                            in0=gb_sb[li][:, F:2 * F],
                                            in1=mu2[:], op=ALU.mult)
                    nc.vector.tensor_tensor(out=st[:, F:2 * F], in0=mu2[:],
                                            in1=tmp[:, 0:F], op=ALU.subtract)
                    # broadcast to stc[128, 2] via selT matmuls
                    pss = psA.tile([128, 512], f32, tag="a", name="pss")
                    nc.tensor.transpose(pss[:F, 0:1], st[:, 0:F],
                                        ident_f[:1, :1])
                    nc.tensor.transpose(pss[:F, 1:2], st[:, F:2 * F],
                                        ident_f[:1, :1])
                    stT = miscp.tile([128, 2], f32, tag="stT")
                    nc.vector.tensor_copy(stT[:F, :], pss[:F, 0:2])
                    nc.tensor.matmul(pss[:, 2:4], selT_sb[F],
                                     stT[:F, 0:2], start=True, stop=True)
                    stc = miscp.tile([128, 2], f32, tag=f"stc{ar_idx}")
                    nc.vector.tensor_copy(stc[:], pss[:, 2:4])
                    ar_idx += 1
                    stc_prev = stc
                    # fold s into next layer's weights
                    nxt = CFGS[li + 1].name
                    for w in "ABC":
                        nc.scalar.activation(W_sb[f"{w}{nxt}"], W_sb[f"{w}{nxt}"],
                                             AF.Copy, scale=stc[:, 0:1])
                    if not CFGS[li + 1].up4:
                        # bias-relu pass on XFn: 3-lane DVE/Act/Pool split
                        # (Pool is idle here: its collective already ran)
                        xv = XFn[:].rearrange("p (g v) -> p g v", v=V)
                        for g in range(cfg.nGp):
                            if g % 4 != 3:
                                nc.vector.tensor_scalar(
                                    out=xv[:, g], in0=xv[:, g],
                                    scalar1=stc[:, 1:2], scalar2=0.0,
                                    op0=ALU.add, op1=ALU.max)
                            else:
                                nc.scalar.activation(
                                    xv[:, g], xv[:, g], AF.Relu,
                                    bias=stc[:, 1:2])
                    XF_cur = XFn

            # ---- store output (first 3072 cols sent during c3) ----
            nc.sync.dma_start(ydram[:, 3072:], och[:, 3072:])

    nc.compile()
    return nc


def kernel(**inputs):
    import sys
    for p in ("/opt/trn_rl_repo", "/opt/trn_rl_repo/concourse"):
        if p not in sys.path:
            sys.path.insert(0, p)
    from concourse.bass_utils import run_bass_kernel_spmd

    host = _build_host(inputs)
    woffs = host.pop("_woffs")

    key = "nc"
    if key not in _CACHE:
        _CACHE[key] = _build_nc(woffs)
    nc = _CACHE[key]

    in_maps = []
    for c in range(NCORES):
        m = {k: v for k, v in host.items() if k != "xT"}
        m["xT"] = np.ascontiguousarray(host["xT"][:, c * BL:(c + 1) * BL])
        in_maps.append(m)
    res = run_bass_kernel_spmd(nc, in_maps, core_ids=list(range(NCORES)))
    P2 = _perm2()
    out = np.empty((B, 1280, 3), np.float32)
    for c in range(NCORES):
        y = res.results[c]["y"].reshape(BL, 1280, 3)
        out[c * BL:(c + 1) * BL][:, P2, :] = y
    return out


if __name__ == "__main__":
    import reference as R
    inp = R.setup_inputs()
    inp = {k: np.asarray(v) for k, v in inp.items()}
    act = kernel(**inp)
    exp = np.asarray(R.reference(**inp))
    err = np.linalg.norm(act - exp) / np.linalg.norm(exp)
    print("Relative error:", err)
